# revision 24
# baseline (speedup 1.0000x reference)
"""NonLocalBlock (B=8, C=256, HW=64x64) Trainium2 kernel.

Data-parallel over batch: one sample per NeuronCore (8 cores).
Per core (everything on-chip; the [N,N]=67MB attention matrix never
touches HBM):

  x [C=256, N=4096] fp16 --DMA--> SBUF (serves logit path, g path, residual)
  theta = w_theta @ x + b_theta    [O=128, N]  fp16   (PE fp16)
  phi   = w_phi   @ x + b_phi      [O=128, N]  fp16
  gt    = (w_g @ x)^T              [N, O] bf16 chunks + ones column
  per 512-wide n-tile (iteration it handles tile nt=it plus leftovers of
  tile pv=it-1):
    S^T chunks [m=128, n=512] = phi_chunk^T . theta_tile  (PE fp16 -> PSUM)
    P^T = exp(S^T): 3-chunk groups alternate between ScalarE (exact EXP
        ACTIVATE, PSUM->SBUF bf16) and DVE (Schraudolph fast-exp: one
        tensor_scalar mult-add producing int16 bits that ARE the bf16
        exp approximation, ~3% rel err - calibrated against the 2e-2
        gate). Splitting exp across two engines removes ScalarE as the
        pipeline pacer (exp is 109us of ScalarE work if unsplit).
    y[s] [128, 129] += P^T_chunk^T . [gt_chunk | ones]    (PE bf16)
        col 128 accumulates the softmax row-sum for free.
        Slices s=0,1 of tile nt run in iteration it lagging the S^T
        pipeline by TWO groups; slices s=2,3 of tile pv are front-loaded
        into positions 0..5 (all their P^T is ready), so the
        y-accumulator banks recycle half an iteration before their next
        writer.
    ynorm = y[:, :128] * (1/y[:,128])    (DVE)
    yt[o, n-sub] = DMA-xbar-transpose(ynorm) on the sync queue; tile 7
        instead transposes on the then-idle PE (is_transpose matmul vs
        an identity, staged through the spare tail of the y PSUM bank).
    z = w_out^T . yt at position 10 of iteration pv+1; bn on ScalarE
        (ACTIVATE Identity with per-partition scale+bias APs); residual
        add on GpSimd (DVE for tile 7); out DMA split across sync/gpsimd
        queues.
       (b_g is folded into bn_shift' on the host: w_out @ b_g is constant)

Prologue (restructured vs the 169.8us version): iteration 0 of the
attention loop is MERGED with the projection pipeline. x streams as 8 x
256KB quarter-DMAs (1024 cols => 2KB per-partition lines for full DMA
rate) split across the sync/gpsimd queues; weights stream as per-half
tiles on the scalar queue ordered by first use (split tiles so the
first matmul depends on exactly the two 32KB DMAs it needs, not all
consts). The projection of block b and the S^T/exp/mm2 groups of tile 0
interleave on the PE as soon as their inputs land, so the PE starts
~9.4us (vs 12.5us: first matmul gated on every const DMA) and has no
DMA-paced idle gaps during the x stream. PSUM during iteration 0:
theta/phi projections double-buffer through 2 one-bank tiles, g through
2 one-bank tiles, S^T groups single-buffer in one 3-bank slot (exp of
every iteration-0 group is split across ScalarE+DVE halves so the
single-buffered slot turns around in ~0.9us while the PE projects the
next block), y01 packs two accumulators in 1 bank.  After iteration 0
the projection pools close and the steady-state ring (2x3-bank S^T
ring + 2 y-accumulator banks) takes the full 8 banks.
"""
import os
import sys

sys.path.insert(0, "/opt/trn_rl_repo")

import numpy as np
import ml_dtypes

import concourse.bass as bass
import concourse.bacc as bacc
import concourse.mybir as mybir
import concourse.tile as tile
from concourse.tile import add_dep_helper
from concourse.bass_utils import run_bass_kernel_spmd

F32 = mybir.dt.float32
F16 = mybir.dt.float16
BF16 = mybir.dt.bfloat16
I16 = mybir.dt.int16
ADD = mybir.AluOpType.add
MULT = mybir.AluOpType.mult
EXP = mybir.ActivationFunctionType.Exp
IDENT = mybir.ActivationFunctionType.Identity

B, C, O, N = 8, 256, 128, 4096
NT = 512
N_TILES = N // NT            # 8
M_CHUNKS = N // 128          # 32
# S^T groups: chunks per exp ACTIVATE (3 banks => double-buffered in 6)
GROUPS = [(c, min(c + 3, M_CHUNKS)) for c in range(0, M_CHUNKS, 3)]  # 11 groups
# groups whose exp runs as a DVE fast-exp instead of a ScalarE ACTIVATE.
# Strict alternation (g9 on DVE, not ScalarE): two consecutive ScalarE
# groups at the tile end made S^T(g10) wait ~0.9us on exp(g8) every tile.
DVE_GROUPS = {1, 3, 5, 7, 9}
# Schraudolph fast-exp into bf16 bits: i16 = trunc(S*A + Bc); bits are bf16.
# c=0.0436 calibrated for truncation semantics; validated rel err 0.007.
FEXP_A = float(128.0 / np.log(2.0))
FEXP_B = float(128.0 * (127.0 - 0.0436))
BN_EPS = 1e-5
# proj blocks that must be emitted before iteration-0 phase-A group g
# (group g reads phi chunks 3g..3g+2 -> block floor((3g+2)/4)); all 8
# blocks project during phase A (groups 0..6), before the projection
# PSUM pools close
BLOCKS_BEFORE_GROUP = [1, 2, 3, 4, 5, 6, 8]


def build_nc():
    nc = bacc.Bacc()

    xin = nc.dram_tensor("xin", [C, N], F16, kind="ExternalInput")
    wth = nc.dram_tensor("wth", [C, O], F16, kind="ExternalInput")     # w_theta.T
    wph = nc.dram_tensor("wph", [C, O], F16, kind="ExternalInput")     # w_phi.T
    wg = nc.dram_tensor("wg", [C, O], F16, kind="ExternalInput")       # w_g.T
    wout = nc.dram_tensor("wout", [O, C], BF16, kind="ExternalInput")  # w_out.T
    bth = nc.dram_tensor("bth", [O, 1], F32, kind="ExternalInput")
    bph = nc.dram_tensor("bph", [O, 1], F32, kind="ExternalInput")
    identm = nc.dram_tensor("identm", [128, 128], BF16, kind="ExternalInput")
    bnscale = nc.dram_tensor("bnscale", [128, 2], F32, kind="ExternalInput")
    bnshift = nc.dram_tensor("bnshift", [128, 2], F32, kind="ExternalInput")
    out = nc.dram_tensor("out", [C, N], F32, kind="ExternalOutput")

    with tile.TileContext(nc) as tc:
        with tc.tile_pool(name="const", bufs=1) as const, \
             tc.tile_pool(name="yt_pool", bufs=3) as yt_pool, \
             tc.tile_pool(name="small", bufs=6) as small, \
             tc.tile_pool(name="ostage", bufs=6) as ostage, \
             tc.tile_pool(name="pt_pool", bufs=22) as pt_pool:
            xpool = proj = const

            # ---- constants on the scalar HWDGE queue, split per-half so
            # each matmul depends on exactly the 32KB DMA it needs.  Only
            # w_theta's halves are emitted before block 0's theta matmuls:
            # dependency waits bind to the queue's completion count at
            # emission time, so anything emitted earlier would gate the
            # first matmul ----
            wth_sb = [const.tile([128, O], F16, name=f"wth{k}") for k in range(2)]
            wph_sb = [const.tile([128, O], F16, name=f"wph{k}") for k in range(2)]
            wg_sb = [const.tile([128, O], F16, name=f"wg{k}") for k in range(2)]
            bth_sb = const.tile([O, 1], F32)
            bph_sb = const.tile([O, 1], F32)
            for k in range(2):
                nc.scalar.dma_start(wth_sb[k][:], wth[k * 128:(k + 1) * 128, :])
            nc.scalar.dma_start(bth_sb[:], bth[:])
            nc.scalar.dma_start(bph_sb[:], bph[:])
            # wout/bn/ident are first read ~30us in; DMAs emitted after the
            # merged iteration-0 loop
            wout_sb = const.tile([O, C], BF16)
            bnscale_sb = const.tile([128, 2], F32)
            bnshift_sb = const.tile([128, 2], F32)
            ident_sb = const.tile([128, 128], BF16)

            # ---- x: per-(half, block) transfers; ALL tops on the sync
            # queue, ALL bottoms on gpsimd (keeping trigger-instruction
            # occupancy off ScalarE/DVE, which are busy in iteration 0).
            # A single transfer moves at only ~33GB/s (one DMA engine);
            # queue throughput (~100GB/s) comes from concurrent in-flight
            # transfers -- so the latency-critical first blocks are split
            # into small sub-DMAs that the queue processes in parallel.
            # Each block's triggers are emitted only after the previous
            # block's consumers (waits bind to the queue's completion
            # count at emission time) ----
            x_blk = [[xpool.tile([128, NT], F16, name=f"x{k}_{b}")
                      for b in range(N_TILES)] for k in range(2)]
            x_emitted = [False] * N_TILES

            def emit_x_block(b):
                if b >= N_TILES or x_emitted[b]:
                    return
                x_emitted[b] = True
                bsl = slice(b * NT, (b + 1) * NT)
                nc.sync.dma_start(x_blk[0][b][:], xin[0:128, bsl])
                nc.gpsimd.dma_start(x_blk[1][b][:], xin[128:256, bsl])

            def xap(k, b):
                return x_blk[k][b]

            theta_h = proj.tile([O, N], F16)
            phi_h = proj.tile([O, N], F16)
            gt_sb = proj.tile([128, M_CHUNKS * (O + 1)], BF16)

            emit_x_block(0)
            # phi/g weights on the sync queue right behind block-0-top: the
            # scalar queue's serial trigger rate (~0.7us each) plus the
            # ~4.5us DMA cold-start would deliver them only at ~15us
            for k in range(2):
                nc.sync.dma_start(wph_sb[k][:], wph[k * 128:(k + 1) * 128, :])
            for k in range(2):
                nc.sync.dma_start(wg_sb[k][:], wg[k * 128:(k + 1) * 128, :])

            # ================= attention =================
            if True:

                def mm2(y, pt, s, c0, c1, pt_c0):
                    # y accumulator slice gets chunks [c0, c1) of P^T tile pt.
                    # Two accumulators share one PSUM bank: only the first
                    # (s even) opens the group (start=True clears the whole
                    # bank); the second writes start=False onto cleared bits.
                    first = None
                    for c in range(c0, c1):
                        i = nc.tensor.matmul(
                            y[:],
                            pt[:, (c - pt_c0) * NT + s * 128:(c - pt_c0) * NT + (s + 1) * 128],
                            gt_sb[:, c * (O + 1):(c + 1) * (O + 1)],
                            start=(c == 0 and s % 2 == 0),
                            stop=(c == M_CHUNKS - 1),
                            skip_group_check=True,
                        )
                        if first is None:
                            first = i
                    return first

                def norm_transpose(y, yt_tile, col, q):
                    recip = small.tile([128, 1], F32)
                    nc.vector.reciprocal(recip[:], y[:, O:O + 1])
                    ynorm = small.tile([128, O], BF16)
                    nc.vector.tensor_scalar_mul(ynorm[:], y[:, 0:O], recip[:])
                    q.dma_start_transpose(yt_tile[:, col:col + 128], ynorm[:])

                def norm_pair_pe(ytile, yt_tile, colbase, copy_eng):
                    # last-tile path: transpose on the (idle) PE instead of the
                    # DMA xbar, staging through the spare tail of the y PSUM
                    # bank; start=True zeroes the whole bank, so the first
                    # transpose must wait for BOTH slices' norm reads.
                    yn = []
                    for lo in (0, 130):
                        recip = small.tile([128, 1], F32)
                        nc.vector.reciprocal(recip[:], ytile[:, lo + O:lo + O + 1])
                        ynorm = small.tile([128, O], BF16)
                        mi = nc.vector.tensor_scalar_mul(
                            ynorm[:], ytile[:, lo:lo + O], recip[:])
                        yn.append((ynorm, mi))
                    dstb = ytile[:, 260:392].bitcast(BF16)
                    prev = None
                    for j, (ynorm, mi) in enumerate(yn):
                        dst = dstb[:, j * 132:j * 132 + 128]
                        im = nc.tensor.matmul(
                            dst, ynorm[:], ident_sb[:], is_transpose=True,
                            start=(j == 0), stop=True, skip_group_check=True,
                        )
                        if j == 0:
                            add_dep_helper(im.ins, yn[1][1].ins, sync=True,
                                           reason="bank clear only after all norm reads")
                        else:
                            add_dep_helper(im.ins, prev.ins, sync=False,
                                           reason="bank-pack: clear before second write")
                        prev = im
                        if copy_eng is nc.scalar:
                            nc.scalar.copy(
                                yt_tile[:, colbase + j * 128:colbase + (j + 1) * 128], dst)
                        else:
                            copy_eng.tensor_copy(
                                yt_tile[:, colbase + j * 128:colbase + (j + 1) * 128], dst)

                def bn_res_dma(src_ap, ct, t, off, width, bn_eng=None):
                    # bn: ACTIVATE Identity with per-partition scale+bias APs
                    # (ScalarE), or the same affine on DVE via tensor_scalar
                    # when ScalarE is the pacer; residual add on GpSimd (DVE
                    # for the last tile); out DMA split across queues.
                    obn = ostage.tile([128, width], F32)
                    if bn_eng is nc.vector:
                        nc.vector.tensor_scalar(
                            obn[:], src_ap,
                            bnscale_sb[:, ct:ct + 1], bnshift_sb[:, ct:ct + 1],
                            op0=MULT, op1=ADD,
                        )
                    else:
                        nc.scalar.activation(
                            obn[:], src_ap, IDENT,
                            bias=bnshift_sb[:, ct:ct + 1], scale=bnscale_sb[:, ct:ct + 1],
                        )
                    ores = ostage.tile([128, width], F32, name="ores")
                    eng = nc.vector if t == N_TILES - 1 else nc.gpsimd
                    eng.tensor_tensor(
                        ores[:], obn[:], xap(ct, t)[:, off:off + width], op=ADD
                    )
                    # last tile: keep everything on sync so the gpsimd DMA
                    # ring has nothing late to drain in the epilogue
                    q = nc.sync if (ct == 0 or t == N_TILES - 1) else nc.gpsimd
                    q.dma_start(
                        out[ct * 128:(ct + 1) * 128, t * NT + off:t * NT + off + width],
                        ores[:])

                def y23_groups(pos):
                    # front-load the 11 groups of the previous tile's y23 mm2
                    # into positions 0..5 (their P^T tiles are all ready)
                    if pos < 5:
                        return [2 * pos, 2 * pos + 1]
                    if pos == 5:
                        return [10]
                    return []

                n_grps = len(GROUPS)
                last = N_TILES - 1
                yts = {}
                yts[0] = yt_pool.tile([O, NT], BF16, name="yt0")

                # Two 4-bank PSUM pools, each holding one 3-bank S^T slot
                # (tag "st") and one y-accumulator bank (tag "ya"); the S^T
                # ring alternates between them.  During iteration-0 phase A
                # only ps0 exists (single-buffered) next to the projection
                # pools; phase B onward alternates ps0/psB with no pool
                # boundary into the steady state.
                st_pools = [None, None]
                ya_pools = [None, None]
                st_idx = [0]

                def st_alloc(width=3 * NT, name="st"):
                    p = st_pools[st_idx[0] % 2] or st_pools[0]
                    st_idx[0] += 1
                    return p.tile([128, width], F32, name=name, tag="st")

                # ---- iteration 0 phase A: groups 0..6 interleaved with all
                # 8 projection blocks as their DMAs land; PSUM: ps0 4 banks
                # (S^T slot + y01) + theta/phi 2 + g 2 = 8 ----
                import contextlib as _ctxlib
                _ps0_stk = _ctxlib.ExitStack()
                ps0 = _ps0_stk.enter_context(
                    tc.tile_pool(name="ps0", bufs=1, space="PSUM"))
                with tc.tile_pool(name="pp", bufs=2, space="PSUM") as pp, \
                     tc.tile_pool(name="gp_ps", bufs=2, space="PSUM") as gp_ps:
                    st_pools[0] = ya_pools[0] = ps0

                    proj_done = [0]
                    exp_warm = small.tile([O, 1], F32)

                    def emit_proj_block(b):
                        # theta/phi/g channel projections for 512-col block b.
                        # blocks 0-1: biases on DVE (ScalarE is loading the
                        # EXP table); later blocks alternate engines.
                        bsl = slice(b * NT, (b + 1) * NT)
                        pth = pp.tile([128, NT], F32, name="pth", tag="pp")
                        for k in range(2):
                            nc.tensor.matmul(
                                pth[:], wth_sb[k][:], xap(k, b),
                                start=(k == 0), stop=(k == 1),
                            )
                        if b == 0:
                            # preload the EXP table before the first real exp
                            nc.scalar.activation(exp_warm[:], bth_sb[:], EXP)
                        if b < 2 or b % 2 == 1:
                            nc.vector.tensor_scalar_add(theta_h[:, bsl], pth[:], bth_sb[:])
                        else:
                            nc.scalar.activation(theta_h[:, bsl], pth[:], IDENT, bias=bth_sb[:])
                        pph = pp.tile([128, NT], F32, name="pph", tag="pp")
                        for k in range(2):
                            nc.tensor.matmul(
                                pph[:], wph_sb[k][:], xap(k, b),
                                start=(k == 0), stop=(k == 1),
                            )
                        if b < 2 or b % 2 == 0:
                            nc.vector.tensor_scalar_add(phi_h[:, bsl], pph[:], bph_sb[:])
                        else:
                            nc.scalar.activation(phi_h[:, bsl], pph[:], IDENT, bias=bph_sb[:])
                        for c in range(4 * b, 4 * b + 4):
                            gpc = gp_ps.tile([128, O], F32, name="gpc")
                            q = (c % 4) * 128
                            for k in range(2):
                                nc.tensor.matmul(
                                    gpc[:],
                                    xap(k, b)[:, q:q + 128],
                                    wg_sb[k][:],
                                    start=(k == 0), stop=(k == 1),
                                )
                            base = c * (O + 1)
                            if c % 2 == 0:
                                nc.vector.tensor_copy(gt_sb[:, base:base + O], gpc[:])
                            else:
                                nc.scalar.copy(gt_sb[:, base:base + O], gpc[:])
                            nc.gpsimd.memset(
                                gt_sb[:, base + O:base + O + 1], 1.0)
                        emit_x_block(b + 1)
                        proj_done[0] = b + 1

                    y01 = ps0.tile([128, 512], F32, name="y01", tag="ya")
                    pts_prev = []

                    def it0_group(g):
                        c0, c1 = GROUPS[g]
                        w = (c1 - c0) * NT
                        st = st_alloc()
                        for c in range(c0, c1):
                            nc.tensor.matmul(
                                st[:, (c - c0) * NT:(c - c0 + 1) * NT],
                                phi_h[:, c * 128:(c + 1) * 128],
                                theta_h[:, 0:NT],
                                start=True, stop=True,
                            )
                        pt = pt_pool.tile([128, 3 * NT], BF16, name="pt")
                        # split every iteration-0 group's exp across both
                        # engines: in phase A the single-buffered S^T slot
                        # then turns around in ~0.9us
                        hw = NT + NT // 2 if w == 3 * NT else w // 2
                        nc.scalar.activation(pt[:, :hw], st[:, :hw], EXP)
                        nc.vector.tensor_scalar(
                            pt[:, hw:w].bitcast(I16), st[:, hw:w],
                            FEXP_A, FEXP_B, op0=MULT, op1=ADD,
                        )
                        pts_prev.append(pt)
                        if g > 1:
                            pc0, pc1 = GROUPS[g - 2]
                            ia = mm2(y01[:, 0:O + 1], pts_prev[g - 2], 0, pc0, pc1, pc0)
                            ib = mm2(y01[:, 130:259], pts_prev[g - 2], 1, pc0, pc1, pc0)
                            if pc0 == 0:
                                add_dep_helper(ib.ins, ia.ins, sync=False,
                                               reason="bank-pack: clear before first write")

                    for g in range(7):
                        while proj_done[0] < BLOCKS_BEFORE_GROUP[g]:
                            emit_proj_block(proj_done[0])
                        it0_group(g)

                # projection pools closed; phase B (groups 7..10) and the
                # steady state run with the st ring alternating ps0/psB
                with tc.tile_pool(name="psB", bufs=1, space="PSUM") as psB:
                    st_pools[1] = ya_pools[1] = psB
                    for g in range(7, n_grps):
                        it0_group(g)
                    for gg in (n_grps - 2, n_grps - 1):
                        c0, c1 = GROUPS[gg]
                        mm2(y01[:, 0:O + 1], pts_prev[gg], 0, c0, c1, c0)
                        mm2(y01[:, 130:259], pts_prev[gg], 1, c0, c1, c0)
                    norm_transpose(y01[:, 0:O + 1], yts[0], 0, nc.sync)
                    norm_transpose(y01[:, 130:259], yts[0], 128, nc.sync)

                    nc.scalar.dma_start(wout_sb[:], wout[:])
                    nc.scalar.dma_start(bnscale_sb[:], bnscale[:])
                    nc.scalar.dma_start(bnshift_sb[:], bnshift[:])
                    nc.scalar.dma_start(ident_sb[:], identm[:])

                    # ---- steady state: iterations 1..N_TILES ----

                    def out_proj_mm_full(t, yt_tile):
                        o01 = st_alloc(1024, "o01")
                        for ct in range(2):
                            nc.tensor.matmul(
                                o01[:, ct * NT:(ct + 1) * NT],
                                wout_sb[:, ct * 128:(ct + 1) * 128],
                                yt_tile[:],
                                start=True, stop=True,
                                skip_group_check=True,
                            )
                        return [(o01[:, ct * NT:(ct + 1) * NT], ct, t, 0, NT)
                                for ct in range(2)]

                    def out_proj_mm_half(t, h, yt_tile):
                        oh = st_alloc(512, "oh")
                        csl = slice(h * 256, (h + 1) * 256)
                        for ct in range(2):
                            nc.tensor.matmul(
                                oh[:, ct * 256:(ct + 1) * 256],
                                wout_sb[:, ct * 128:(ct + 1) * 128],
                                yt_tile[:, csl],
                                start=True, stop=True,
                                skip_group_check=True,
                            )
                        return [(oh[:, ct * 256:(ct + 1) * 256], ct, t, h * 256, 256)
                                for ct in range(2)]

                    for it in range(1, N_TILES + 1):
                        nt = it if it < N_TILES else None
                        pv = it - 1
                        if nt is not None:
                            ntsl = slice(nt * NT, (nt + 1) * NT)
                            y01 = ya_pools[nt % 2].tile([128, 392], F32, name="y01", tag="ya")
                            yts[nt] = yt_pool.tile([O, NT], BF16, name=f"yt{nt}")
                        y23 = ya_pools[pv % 2].tile([128, 392], F32, name="y23", tag="ya")
                        pts_cur = []
                        bns = []
                        for g in range(n_grps):
                            if g == 0:
                                # position 0: previous tile's y23 mm2 first
                                # (deps long met) so the PE has work while the
                                # previous out-projection's bn reads free the
                                # S^T-ring slot
                                for gg in y23_groups(0):
                                    c0, c1 = GROUPS[gg]
                                    ia = mm2(y23[:, 0:O + 1], pts_prev[gg], 2, c0, c1, c0)
                                    ib = mm2(y23[:, 130:259], pts_prev[gg], 3, c0, c1, c0)
                                    if c0 == 0:
                                        add_dep_helper(ib.ins, ia.ins, sync=False,
                                                       reason="bank-pack: clear before first write")
                            if nt is not None:
                                c0, c1 = GROUPS[g]
                                w = (c1 - c0) * NT
                                st = st_alloc()
                                for c in range(c0, c1):
                                    nc.tensor.matmul(
                                        st[:, (c - c0) * NT:(c - c0 + 1) * NT],
                                        phi_h[:, c * 128:(c + 1) * 128],
                                        theta_h[:, ntsl],
                                        start=True, stop=True,
                                    )
                                pt = pt_pool.tile([128, 3 * NT], BF16)
                                if g == n_grps - 1:
                                    # the last group's exp gates both the final
                                    # y01 mm2s and (via the S^T ring slot) the
                                    # next tile's first S^T: split its 2 chunks
                                    # across both engines to halve the latency
                                    nc.scalar.activation(pt[:, :NT], st[:, :NT], EXP)
                                    nc.vector.tensor_scalar(
                                        pt[:, NT:w].bitcast(I16), st[:, NT:w],
                                        FEXP_A, FEXP_B, op0=MULT, op1=ADD,
                                    )
                                elif g in DVE_GROUPS:
                                    nc.vector.tensor_scalar(
                                        pt[:, :w].bitcast(I16), st[:, :w],
                                        FEXP_A, FEXP_B, op0=MULT, op1=ADD,
                                    )
                                else:
                                    nc.scalar.activation(pt[:, :w], st[:, :w], EXP)
                                pts_cur.append(pt)
                                if g > 1:
                                    # y01 mm2 lags TWO groups behind S^T: exp(g)
                                    # then has ~2 pipeline positions before its
                                    # consumer instead of 1
                                    pc0, pc1 = GROUPS[g - 2]
                                    ia = mm2(y01[:, 0:O + 1], pts_cur[g - 2], 0, pc0, pc1, pc0)
                                    ib = mm2(y01[:, 130:259], pts_cur[g - 2], 1, pc0, pc1, pc0)
                                    if pc0 == 0:
                                        add_dep_helper(ib.ins, ia.ins, sync=False,
                                                       reason="bank-pack: clear before first write")
                            if g > 0:
                                for gg in y23_groups(g):
                                    c0, c1 = GROUPS[gg]
                                    ia = mm2(y23[:, 0:O + 1], pts_prev[gg], 2, c0, c1, c0)
                                    ib = mm2(y23[:, 130:259], pts_prev[gg], 3, c0, c1, c0)
                                    if c0 == 0:
                                        add_dep_helper(ib.ins, ia.ins, sync=False,
                                                       reason="bank-pack: clear before first write")
                            if g == 6:
                                if pv == last:
                                    # tail: first half of tile 7's output
                                    # projection straight after the final mm2
                                    # burst; its bn chains emitted immediately
                                    # (ct0 on DVE, ct1 on ScalarE) so they
                                    # overlap the norms/transposes
                                    for ai, args in enumerate(out_proj_mm_half(pv, 0, yts[pv])):
                                        bn_res_dma(*args, bn_eng=nc.vector if ai == 0 else None)
                                    norm_pair_pe(y23, yts[pv], 256, nc.scalar)
                                else:
                                    norm_transpose(y23[:, 0:O + 1], yts[pv], 256, nc.sync)
                                    norm_transpose(y23[:, 130:259], yts[pv], 384, nc.sync)
                            if g == 10:
                                if pv < last:
                                    bns += out_proj_mm_full(pv, yts[pv])
                                else:
                                    for ai, args in enumerate(out_proj_mm_half(pv, 1, yts[pv])):
                                        bn_res_dma(*args, bn_eng=nc.vector if ai == 0 else None)
                        if nt is not None:
                            for gg in (n_grps - 2, n_grps - 1):
                                c0, c1 = GROUPS[gg]
                                mm2(y01[:, 0:O + 1], pts_cur[gg], 0, c0, c1, c0)
                                mm2(y01[:, 130:259], pts_cur[gg], 1, c0, c1, c0)
                        for args in bns:
                            bn_res_dma(*args)
                        if nt is not None:
                            if nt == last:
                                # copies on DVE: scalar is still draining
                                # exp/bn residue here, and these copies gate h0
                                norm_pair_pe(y01, yts[nt], 0, nc.vector)
                            else:
                                norm_transpose(y01[:, 0:O + 1], yts[nt], 0, nc.sync)
                                norm_transpose(y01[:, 130:259], yts[nt], 128, nc.sync)
                        pts_prev = pts_cur
                _ps0_stk.close()

    nc.finalize()
    return nc


_NC_CACHE = None


def _get_nc():
    global _NC_CACHE
    if _NC_CACHE is None:
        _NC_CACHE = build_nc()
    return _NC_CACHE


def _prepare_in_maps(inputs):
    x = np.ascontiguousarray(np.asarray(inputs["x"], dtype=np.float32)).reshape(B, C, N)
    xh = x.astype(np.float16)
    wth = np.ascontiguousarray(np.asarray(inputs["w_theta"], np.float32).T).astype(np.float16)
    wph = np.ascontiguousarray(np.asarray(inputs["w_phi"], np.float32).T).astype(np.float16)
    wg = np.ascontiguousarray(np.asarray(inputs["w_g"], np.float32).T).astype(np.float16)
    w_out = np.asarray(inputs["w_out"], np.float32)
    wout = np.ascontiguousarray(w_out.T).astype(ml_dtypes.bfloat16)
    bth = np.asarray(inputs["b_theta"], np.float32).reshape(O, 1)
    bph = np.asarray(inputs["b_phi"], np.float32).reshape(O, 1)
    inv = np.asarray(inputs["bn_gamma"], np.float32) / np.sqrt(
        np.asarray(inputs["bn_var"], np.float32) + BN_EPS)
    shift = (np.asarray(inputs["b_out"], np.float32) * inv
             + np.asarray(inputs["bn_beta"], np.float32)
             - np.asarray(inputs["bn_mean"], np.float32) * inv)
    # fold the g-branch bias through the output projection: softmax rows sum
    # to 1, so attn @ (g + b_g) = attn @ g + b_g, and w_out @ b_g is constant
    wob = wout.astype(np.float32) .T @ np.asarray(inputs["b_g"], np.float32)
    shift = shift + inv * wob
    bnscale = np.ascontiguousarray(inv.reshape(2, 128).T)
    bnshift = np.ascontiguousarray(shift.reshape(2, 128).T)

    shared = dict(wth=wth, wph=wph, wg=wg, wout=wout, bth=bth, bph=bph,
                  bnscale=bnscale, bnshift=bnshift,
                  identm=np.eye(128, dtype=ml_dtypes.bfloat16))
    return [dict(shared, xin=np.ascontiguousarray(xh[b])) for b in range(B)]


def _install_ntff_shim():
    """This image's antenv lacks axon_hooks; provide it from trn_boot's
    ctypes implementation so trace=True can capture NTFF profiles."""
    import types
    try:
        import antenv.axon_hooks  # noqa: F401
        return
    except ImportError:
        pass
    if "/root/.axon_site" not in sys.path:
        sys.path.insert(0, "/root/.axon_site")
    from trn_agent_boot.trn_boot import _ntff_profile_via_ctypes
    hook = _ntff_profile_via_ctypes("/opt/axon/libaxon_pjrt.so")
    m = types.ModuleType("antenv.axon_hooks")
    m.get_axon_ntff_profile_hook = lambda: hook
    m.set_axon_ntff_profile_hook = lambda h: None
    sys.modules["antenv.axon_hooks"] = m


def run(inputs, trace=False):
    if trace:
        _install_ntff_shim()
    nc = _get_nc()
    in_maps = _prepare_in_maps(inputs)
    res = run_bass_kernel_spmd(nc, in_maps, list(range(B)), trace=trace)
    outs = np.stack([res.results[b]["out"] for b in range(B)])
    return outs.reshape(B, C, 64, 64), res


def kernel(**inputs) -> np.ndarray:
    out, _ = run(inputs)
    return out


if __name__ == "__main__":
    # quick CoreSim check of one core
    from concourse import bass_interp
    rng = np.random.default_rng(0)
    fake = {
        "x": rng.standard_normal((B, C, 64, 64)).astype(np.float32),
        "w_theta": (rng.standard_normal((O, C)) * 0.05).astype(np.float32),
        "b_theta": (rng.standard_normal(O) * 0.05).astype(np.float32),
        "w_phi": (rng.standard_normal((O, C)) * 0.05).astype(np.float32),
        "b_phi": (rng.standard_normal(O) * 0.05).astype(np.float32),
        "w_g": (rng.standard_normal((O, C)) * 0.05).astype(np.float32),
        "b_g": (rng.standard_normal(O) * 0.05).astype(np.float32),
        "w_out": (rng.standard_normal((C, O)) * 0.05).astype(np.float32),
        "b_out": (rng.standard_normal(C) * 0.05).astype(np.float32),
        "bn_gamma": rng.standard_normal(C).astype(np.float32),
        "bn_beta": rng.standard_normal(C).astype(np.float32),
        "bn_mean": rng.standard_normal(C).astype(np.float32),
        "bn_var": rng.uniform(0.5, 1.5, C).astype(np.float32),
    }
    nc = _get_nc()
    in_maps = _prepare_in_maps(fake)
    sim = bass_interp.CoreSim(nc)
    for k, v in in_maps[0].items():
        sim.tensor(k)[:] = v
    sim.simulate()
    got = np.asarray(sim.tensor("out"))

    x0 = fake["x"][0].reshape(C, N)
    th = fake["w_theta"] @ x0 + fake["b_theta"][:, None]
    ph = fake["w_phi"] @ x0 + fake["b_phi"][:, None]
    gg = fake["w_g"] @ x0 + fake["b_g"][:, None]
    s = th.T @ ph
    p = np.exp(s - s.max(1, keepdims=True))
    a = p / p.sum(1, keepdims=True)
    yy = a @ gg.T
    wy = fake["w_out"] @ yy.T + fake["b_out"][:, None]
    inv = fake["bn_gamma"] / np.sqrt(fake["bn_var"] + BN_EPS)
    bn = wy * inv[:, None] + (fake["bn_beta"] - fake["bn_mean"] * inv)[:, None]
    want = x0 + bn
    err = np.abs(got - want).max()
    print("CoreSim absmax err:", err, "rel:", err / np.abs(want).max())


# revision 25
# speedup vs baseline: 1.0567x; 1.0567x over previous
"""NonLocalBlock (B=8, C=256, HW=64x64) Trainium2 kernel.

Data-parallel over batch: one sample per NeuronCore (8 cores).
Per core (everything on-chip; the [N,N]=67MB attention matrix never
touches HBM):

  x [C=256, N=4096] fp16 --DMA--> SBUF (serves logit path, g path, residual)
  theta = w_theta @ x + b_theta    [O=128, N]  fp16   (PE fp16)
  phi   = w_phi   @ x + b_phi      [O=128, N]  fp16
  gt    = (w_g @ x)^T              [N, O] bf16 chunks + ones column
  per 512-wide n-tile (iteration it handles tile nt=it plus leftovers of
  tile pv=it-1):
    S^T chunks [m=128, n=512] = phi_chunk^T . theta_tile  (PE fp16 -> PSUM)
    P^T = exp(S^T): 3-chunk groups alternate between ScalarE (exact EXP
        ACTIVATE, PSUM->SBUF bf16) and DVE (Schraudolph fast-exp: one
        tensor_scalar mult-add producing int16 bits that ARE the bf16
        exp approximation, ~3% rel err - calibrated against the 2e-2
        gate). Splitting exp across two engines removes ScalarE as the
        pipeline pacer (exp is 109us of ScalarE work if unsplit).
    y[s] [128, 129] += P^T_chunk^T . [gt_chunk | ones]    (PE bf16)
        col 128 accumulates the softmax row-sum for free.
        Slices s=0,1 of tile nt run in iteration it lagging the S^T
        pipeline by TWO groups; slices s=2,3 of tile pv are front-loaded
        into positions 0..5 (all their P^T is ready), so the
        y-accumulator banks recycle half an iteration before their next
        writer.
    ynorm = y[:, :128] * (1/y[:,128])    (DVE)
    yt[o, n-sub] = DMA-xbar-transpose(ynorm) on the sync queue; tile 7
        instead transposes on the then-idle PE (is_transpose matmul vs
        an identity, staged through the spare tail of the y PSUM bank).
    z = w_out^T . yt at position 10 of iteration pv+1; bn on ScalarE
        (ACTIVATE Identity with per-partition scale+bias APs); residual
        add on GpSimd (DVE for tile 7); out DMA split across sync/gpsimd
        queues.
       (b_g is folded into bn_shift' on the host: w_out @ b_g is constant)

Prologue (restructured vs the 169.8us version): iteration 0 of the
attention loop is MERGED with the projection pipeline. x streams as 8 x
256KB quarter-DMAs (1024 cols => 2KB per-partition lines for full DMA
rate) split across the sync/gpsimd queues; weights stream as per-half
tiles on the scalar queue ordered by first use (split tiles so the
first matmul depends on exactly the two 32KB DMAs it needs, not all
consts). The projection of block b and the S^T/exp/mm2 groups of tile 0
interleave on the PE as soon as their inputs land, so the PE starts
~9.4us (vs 12.5us: first matmul gated on every const DMA) and has no
DMA-paced idle gaps during the x stream. PSUM during iteration 0:
theta/phi projections double-buffer through 2 one-bank tiles, g through
2 one-bank tiles, S^T groups single-buffer in one 3-bank slot (exp of
every iteration-0 group is split across ScalarE+DVE halves so the
single-buffered slot turns around in ~0.9us while the PE projects the
next block), y01 packs two accumulators in 1 bank.  After iteration 0
the projection pools close and the steady-state ring (2x3-bank S^T
ring + 2 y-accumulator banks) takes the full 8 banks.
"""
import os
import sys

sys.path.insert(0, "/opt/trn_rl_repo")

import numpy as np
import ml_dtypes

import concourse.bass as bass
import concourse.bacc as bacc
import concourse.mybir as mybir
import concourse.tile as tile
from concourse.tile import add_dep_helper
from concourse.bass_utils import run_bass_kernel_spmd

F32 = mybir.dt.float32
F16 = mybir.dt.float16
BF16 = mybir.dt.bfloat16
I16 = mybir.dt.int16
ADD = mybir.AluOpType.add
MULT = mybir.AluOpType.mult
EXP = mybir.ActivationFunctionType.Exp
IDENT = mybir.ActivationFunctionType.Identity

B, C, O, N = 8, 256, 128, 4096
NT = 512
N_TILES = N // NT            # 8
M_CHUNKS = N // 128          # 32
# S^T groups: chunks per exp ACTIVATE (3 banks => double-buffered in 6)
GROUPS = [(c, min(c + 3, M_CHUNKS)) for c in range(0, M_CHUNKS, 3)]  # 11 groups
# groups whose exp runs as a DVE fast-exp instead of a ScalarE ACTIVATE.
# Strict alternation (g9 on DVE, not ScalarE): two consecutive ScalarE
# groups at the tile end made S^T(g10) wait ~0.9us on exp(g8) every tile.
DVE_GROUPS = {1, 3, 5, 7, 9}
# Schraudolph fast-exp into bf16 bits: i16 = trunc(S*A + Bc); bits are bf16.
# c=0.0436 calibrated for truncation semantics; validated rel err 0.007.
FEXP_A = float(128.0 / np.log(2.0))
FEXP_B = float(128.0 * (127.0 - 0.0436))
BN_EPS = 1e-5
# proj blocks that must be emitted before iteration-0 group g: group g
# reads phi chunks 3g..3g+2 -> block floor((3g+2)/4)
BLOCKS_BEFORE_GROUP = [1, 2, 3, 3, 4, 5, 6, 6, 7, 8, 8]


def build_nc():
    nc = bacc.Bacc()

    xin = nc.dram_tensor("xin", [C, N], F16, kind="ExternalInput")
    wth = nc.dram_tensor("wth", [C, O], F16, kind="ExternalInput")     # w_theta.T
    wph = nc.dram_tensor("wph", [C, O], F16, kind="ExternalInput")     # w_phi.T
    wg = nc.dram_tensor("wg", [C, O], F16, kind="ExternalInput")       # w_g.T
    wout = nc.dram_tensor("wout", [O, C], BF16, kind="ExternalInput")  # w_out.T
    bth = nc.dram_tensor("bth", [O, 1], F32, kind="ExternalInput")
    bph = nc.dram_tensor("bph", [O, 1], F32, kind="ExternalInput")
    identm = nc.dram_tensor("identm", [128, 128], BF16, kind="ExternalInput")
    bnscale = nc.dram_tensor("bnscale", [128, 2], F32, kind="ExternalInput")
    bnshift = nc.dram_tensor("bnshift", [128, 2], F32, kind="ExternalInput")
    out = nc.dram_tensor("out", [C, N], F32, kind="ExternalOutput")

    with tile.TileContext(nc) as tc:
        with tc.tile_pool(name="const", bufs=1) as const, \
             tc.tile_pool(name="yt_pool", bufs=3) as yt_pool, \
             tc.tile_pool(name="small", bufs=6) as small, \
             tc.tile_pool(name="ostage", bufs=6) as ostage, \
             tc.tile_pool(name="pt_pool", bufs=22) as pt_pool:
            xpool = proj = const

            # ---- constants on the scalar HWDGE queue, split per-half so
            # each matmul depends on exactly the 32KB DMA it needs.  Only
            # w_theta's halves are emitted before block 0's theta matmuls:
            # dependency waits bind to the queue's completion count at
            # emission time, so anything emitted earlier would gate the
            # first matmul ----
            wth_sb = [const.tile([128, O], F16, name=f"wth{k}") for k in range(2)]
            wph_sb = [const.tile([128, O], F16, name=f"wph{k}") for k in range(2)]
            wg_sb = [const.tile([128, O], F16, name=f"wg{k}") for k in range(2)]
            bth_sb = const.tile([O, 1], F32)
            bph_sb = const.tile([O, 1], F32)
            for k in range(2):
                nc.scalar.dma_start(wth_sb[k][:], wth[k * 128:(k + 1) * 128, :])
            nc.scalar.dma_start(bth_sb[:], bth[:])
            nc.scalar.dma_start(bph_sb[:], bph[:])
            # wout/bn/ident are first read ~30us in; DMAs emitted after the
            # merged iteration-0 loop
            wout_sb = const.tile([O, C], BF16)
            bnscale_sb = const.tile([128, 2], F32)
            bnshift_sb = const.tile([128, 2], F32)
            ident_sb = const.tile([128, 128], BF16)

            # ---- x: per-(half, block) transfers; ALL tops on the sync
            # queue, ALL bottoms on gpsimd (keeping trigger-instruction
            # occupancy off ScalarE/DVE, which are busy in iteration 0).
            # A single transfer moves at only ~33GB/s (one DMA engine);
            # queue throughput (~100GB/s) comes from concurrent in-flight
            # transfers -- so the latency-critical first blocks are split
            # into small sub-DMAs that the queue processes in parallel.
            # Each block's triggers are emitted only after the previous
            # block's consumers (waits bind to the queue's completion
            # count at emission time) ----
            x_blk = [[xpool.tile([128, NT], F16, name=f"x{k}_{b}")
                      for b in range(N_TILES)] for k in range(2)]
            x_emitted = [False] * N_TILES

            def emit_x_block(b):
                if b >= N_TILES or x_emitted[b]:
                    return
                x_emitted[b] = True
                bsl = slice(b * NT, (b + 1) * NT)
                nc.sync.dma_start(x_blk[0][b][:], xin[0:128, bsl])
                nc.gpsimd.dma_start(x_blk[1][b][:], xin[128:256, bsl])

            def xap(k, b):
                return x_blk[k][b]

            theta_h = proj.tile([O, N], F16)
            phi_h = proj.tile([O, N], F16)
            gt_sb = proj.tile([128, M_CHUNKS * (O + 1)], BF16)

            emit_x_block(0)
            # phi/g weights on the sync queue right behind block-0-top: the
            # scalar queue's serial trigger rate (~0.7us each) plus the
            # ~4.5us DMA cold-start would deliver them only at ~15us
            for k in range(2):
                nc.sync.dma_start(wph_sb[k][:], wph[k * 128:(k + 1) * 128, :])
            for k in range(2):
                nc.sync.dma_start(wg_sb[k][:], wg[k * 128:(k + 1) * 128, :])

            # ================= attention =================
            if True:

                def mm2(y, pt, s, c0, c1, pt_c0):
                    # y accumulator slice gets chunks [c0, c1) of P^T tile pt.
                    # Two accumulators share one PSUM bank: only the first
                    # (s even) opens the group (start=True clears the whole
                    # bank); the second writes start=False onto cleared bits.
                    first = None
                    for c in range(c0, c1):
                        i = nc.tensor.matmul(
                            y[:],
                            pt[:, (c - pt_c0) * NT + s * 128:(c - pt_c0) * NT + (s + 1) * 128],
                            gt_sb[:, c * (O + 1):(c + 1) * (O + 1)],
                            start=(c == 0 and s % 2 == 0),
                            stop=(c == M_CHUNKS - 1),
                            skip_group_check=True,
                        )
                        if first is None:
                            first = i
                    return first

                def norm_transpose(y, yt_tile, col, q):
                    recip = small.tile([128, 1], F32)
                    nc.vector.reciprocal(recip[:], y[:, O:O + 1])
                    ynorm = small.tile([128, O], BF16)
                    nc.vector.tensor_scalar_mul(ynorm[:], y[:, 0:O], recip[:])
                    q.dma_start_transpose(yt_tile[:, col:col + 128], ynorm[:])

                def norm_pair_pe(ytile, yt_tile, colbase, copy_eng):
                    # last-tile path: transpose on the (idle) PE instead of the
                    # DMA xbar, staging through the spare tail of the y PSUM
                    # bank; start=True zeroes the whole bank, so the first
                    # transpose must wait for BOTH slices' norm reads.
                    yn = []
                    for lo in (0, 130):
                        recip = small.tile([128, 1], F32)
                        nc.vector.reciprocal(recip[:], ytile[:, lo + O:lo + O + 1])
                        ynorm = small.tile([128, O], BF16)
                        mi = nc.vector.tensor_scalar_mul(
                            ynorm[:], ytile[:, lo:lo + O], recip[:])
                        yn.append((ynorm, mi))
                    dstb = ytile[:, 260:392].bitcast(BF16)
                    prev = None
                    for j, (ynorm, mi) in enumerate(yn):
                        dst = dstb[:, j * 132:j * 132 + 128]
                        im = nc.tensor.matmul(
                            dst, ynorm[:], ident_sb[:], is_transpose=True,
                            start=(j == 0), stop=True, skip_group_check=True,
                        )
                        if j == 0:
                            add_dep_helper(im.ins, yn[1][1].ins, sync=True,
                                           reason="bank clear only after all norm reads")
                        else:
                            add_dep_helper(im.ins, prev.ins, sync=False,
                                           reason="bank-pack: clear before second write")
                        prev = im
                        if copy_eng is nc.scalar:
                            nc.scalar.copy(
                                yt_tile[:, colbase + j * 128:colbase + (j + 1) * 128], dst)
                        else:
                            copy_eng.tensor_copy(
                                yt_tile[:, colbase + j * 128:colbase + (j + 1) * 128], dst)

                def bn_res_dma(src_ap, ct, t, off, width, bn_eng=None):
                    # bn: ACTIVATE Identity with per-partition scale+bias APs
                    # (ScalarE), or the same affine on DVE via tensor_scalar
                    # when ScalarE is the pacer; residual add on GpSimd (DVE
                    # for the last tile); out DMA split across queues.
                    obn = ostage.tile([128, width], F32)
                    if bn_eng is nc.vector:
                        nc.vector.tensor_scalar(
                            obn[:], src_ap,
                            bnscale_sb[:, ct:ct + 1], bnshift_sb[:, ct:ct + 1],
                            op0=MULT, op1=ADD,
                        )
                    else:
                        nc.scalar.activation(
                            obn[:], src_ap, IDENT,
                            bias=bnshift_sb[:, ct:ct + 1], scale=bnscale_sb[:, ct:ct + 1],
                        )
                    ores = ostage.tile([128, width], F32, name="ores")
                    eng = nc.vector if t == N_TILES - 1 else nc.gpsimd
                    eng.tensor_tensor(
                        ores[:], obn[:], xap(ct, t)[:, off:off + width], op=ADD
                    )
                    # last tile: keep everything on sync so the gpsimd DMA
                    # ring has nothing late to drain in the epilogue
                    q = nc.sync if (ct == 0 or t == N_TILES - 1) else nc.gpsimd
                    q.dma_start(
                        out[ct * 128:(ct + 1) * 128, t * NT + off:t * NT + off + width],
                        ores[:])

                def y23_groups(pos):
                    # front-load the 11 groups of the previous tile's y23 mm2
                    # into positions 0..5 (their P^T tiles are all ready)
                    if pos < 5:
                        return [2 * pos, 2 * pos + 1]
                    if pos == 5:
                        return [10]
                    return []

                n_grps = len(GROUPS)
                last = N_TILES - 1
                yts = {}
                yts[0] = yt_pool.tile([O, NT], BF16, name="yt0")

                # Two 4-bank PSUM pools, each holding one 3-bank S^T slot
                # (tag "st") and one y-accumulator bank (tag "ya"); the S^T
                # ring alternates between them.  During iteration-0 phase A
                # only ps0 exists (single-buffered) next to the projection
                # pools; phase B onward alternates ps0/psB with no pool
                # boundary into the steady state.
                st_pools = [None, None]
                ya_pools = [None, None]
                st_idx = [0]

                def st_alloc(width=3 * NT, name="st"):
                    p = st_pools[st_idx[0] % 2] or st_pools[0]
                    st_idx[0] += 1
                    return p.tile([128, width], F32, name=name, tag="st")

                # ---- iteration 0 phase A: groups 0..6 interleaved with all
                # 8 projection blocks as their DMAs land; PSUM: ps0 4 banks
                # (S^T slot + y01) + theta/phi 2 + g 2 = 8 ----
                with tc.tile_pool(name="ps0", bufs=1, space="PSUM") as ps0, \
                     tc.tile_pool(name="pp", bufs=2, space="PSUM") as pp, \
                     tc.tile_pool(name="gp_ps", bufs=2, space="PSUM") as gp_ps:
                    st_pools[0] = ya_pools[0] = ps0

                    proj_done = [0]
                    exp_warm = small.tile([O, 1], F32)

                    def emit_proj_block(b):
                        # theta/phi/g channel projections for 512-col block b.
                        # blocks 0-1: biases on DVE (ScalarE is loading the
                        # EXP table); later blocks alternate engines.
                        bsl = slice(b * NT, (b + 1) * NT)
                        pth = pp.tile([128, NT], F32, name="pth", tag="pp")
                        for k in range(2):
                            nc.tensor.matmul(
                                pth[:], wth_sb[k][:], xap(k, b),
                                start=(k == 0), stop=(k == 1),
                            )
                        if b == 0:
                            # preload the EXP table before the first real exp
                            nc.scalar.activation(exp_warm[:], bth_sb[:], EXP)
                        if b < 2 or b % 2 == 1:
                            nc.vector.tensor_scalar_add(theta_h[:, bsl], pth[:], bth_sb[:])
                        else:
                            nc.scalar.activation(theta_h[:, bsl], pth[:], IDENT, bias=bth_sb[:])
                        pph = pp.tile([128, NT], F32, name="pph", tag="pp")
                        for k in range(2):
                            nc.tensor.matmul(
                                pph[:], wph_sb[k][:], xap(k, b),
                                start=(k == 0), stop=(k == 1),
                            )
                        if b < 2 or b % 2 == 0:
                            nc.vector.tensor_scalar_add(phi_h[:, bsl], pph[:], bph_sb[:])
                        else:
                            nc.scalar.activation(phi_h[:, bsl], pph[:], IDENT, bias=bph_sb[:])
                        for c in range(4 * b, 4 * b + 4):
                            gpc = gp_ps.tile([128, O], F32, name="gpc")
                            q = (c % 4) * 128
                            for k in range(2):
                                nc.tensor.matmul(
                                    gpc[:],
                                    xap(k, b)[:, q:q + 128],
                                    wg_sb[k][:],
                                    start=(k == 0), stop=(k == 1),
                                )
                            base = c * (O + 1)
                            if c % 2 == 0:
                                nc.vector.tensor_copy(gt_sb[:, base:base + O], gpc[:])
                            else:
                                nc.scalar.copy(gt_sb[:, base:base + O], gpc[:])
                            nc.gpsimd.memset(
                                gt_sb[:, base + O:base + O + 1], 1.0)
                        emit_x_block(b + 1)
                        proj_done[0] = b + 1

                    y01 = ps0.tile([128, 512], F32, name="y01", tag="ya")
                    pts_prev = []

                    def it0_group(g):
                        c0, c1 = GROUPS[g]
                        w = (c1 - c0) * NT
                        st = st_alloc()
                        for c in range(c0, c1):
                            nc.tensor.matmul(
                                st[:, (c - c0) * NT:(c - c0 + 1) * NT],
                                phi_h[:, c * 128:(c + 1) * 128],
                                theta_h[:, 0:NT],
                                start=True, stop=True,
                            )
                        pt = pt_pool.tile([128, 3 * NT], BF16, name="pt")
                        # split every iteration-0 group's exp across both
                        # engines: in phase A the single-buffered S^T slot
                        # then turns around in ~0.9us
                        hw = NT + NT // 2 if w == 3 * NT else w // 2
                        nc.scalar.activation(pt[:, :hw], st[:, :hw], EXP)
                        nc.vector.tensor_scalar(
                            pt[:, hw:w].bitcast(I16), st[:, hw:w],
                            FEXP_A, FEXP_B, op0=MULT, op1=ADD,
                        )
                        pts_prev.append(pt)
                        if g > 1:
                            pc0, pc1 = GROUPS[g - 2]
                            ia = mm2(y01[:, 0:O + 1], pts_prev[g - 2], 0, pc0, pc1, pc0)
                            ib = mm2(y01[:, 130:259], pts_prev[g - 2], 1, pc0, pc1, pc0)
                            if pc0 == 0:
                                add_dep_helper(ib.ins, ia.ins, sync=False,
                                               reason="bank-pack: clear before first write")

                    for g in range(n_grps):
                        while proj_done[0] < BLOCKS_BEFORE_GROUP[g]:
                            emit_proj_block(proj_done[0])
                        it0_group(g)
                    for gg in (n_grps - 2, n_grps - 1):
                        c0, c1 = GROUPS[gg]
                        mm2(y01[:, 0:O + 1], pts_prev[gg], 0, c0, c1, c0)
                        mm2(y01[:, 130:259], pts_prev[gg], 1, c0, c1, c0)
                    norm_transpose(y01[:, 0:O + 1], yts[0], 0, nc.sync)
                    norm_transpose(y01[:, 130:259], yts[0], 128, nc.sync)

                nc.scalar.dma_start(wout_sb[:], wout[:])
                nc.scalar.dma_start(bnscale_sb[:], bnscale[:])
                nc.scalar.dma_start(bnshift_sb[:], bnshift[:])
                nc.scalar.dma_start(ident_sb[:], identm[:])

                # ---- steady state: iterations 1..N_TILES ----
                with tc.tile_pool(name="st_ps", bufs=2, space="PSUM") as st_ps, \
                     tc.tile_pool(name="ya", bufs=2, space="PSUM") as ya:

                    def out_proj_mm_full(t, yt_tile):
                        o01 = st_ps.tile([128, 1024], F32, name="o01", tag="st")
                        for ct in range(2):
                            nc.tensor.matmul(
                                o01[:, ct * NT:(ct + 1) * NT],
                                wout_sb[:, ct * 128:(ct + 1) * 128],
                                yt_tile[:],
                                start=True, stop=True,
                                skip_group_check=True,
                            )
                        return [(o01[:, ct * NT:(ct + 1) * NT], ct, t, 0, NT)
                                for ct in range(2)]

                    def out_proj_mm_half(t, h, yt_tile):
                        oh = st_ps.tile([128, 512], F32, name="oh", tag="st")
                        csl = slice(h * 256, (h + 1) * 256)
                        for ct in range(2):
                            nc.tensor.matmul(
                                oh[:, ct * 256:(ct + 1) * 256],
                                wout_sb[:, ct * 128:(ct + 1) * 128],
                                yt_tile[:, csl],
                                start=True, stop=True,
                                skip_group_check=True,
                            )
                        return [(oh[:, ct * 256:(ct + 1) * 256], ct, t, h * 256, 256)
                                for ct in range(2)]

                    for it in range(1, N_TILES + 1):
                        nt = it if it < N_TILES else None
                        pv = it - 1
                        if nt is not None:
                            ntsl = slice(nt * NT, (nt + 1) * NT)
                            y01 = ya.tile([128, 392], F32, name="y01", tag="ya")
                            yts[nt] = yt_pool.tile([O, NT], BF16, name=f"yt{nt}")
                        y23 = ya.tile([128, 392], F32, name="y23", tag="ya")
                        pts_cur = []
                        bns = []
                        for g in range(n_grps):
                            if g == 0:
                                # position 0: previous tile's y23 mm2 first
                                # (deps long met) so the PE has work while the
                                # previous out-projection's bn reads free the
                                # S^T-ring slot
                                for gg in y23_groups(0):
                                    c0, c1 = GROUPS[gg]
                                    ia = mm2(y23[:, 0:O + 1], pts_prev[gg], 2, c0, c1, c0)
                                    ib = mm2(y23[:, 130:259], pts_prev[gg], 3, c0, c1, c0)
                                    if c0 == 0:
                                        add_dep_helper(ib.ins, ia.ins, sync=False,
                                                       reason="bank-pack: clear before first write")
                            if nt is not None:
                                c0, c1 = GROUPS[g]
                                w = (c1 - c0) * NT
                                st = st_ps.tile([128, 3 * NT], F32, name="st", tag="st")
                                for c in range(c0, c1):
                                    nc.tensor.matmul(
                                        st[:, (c - c0) * NT:(c - c0 + 1) * NT],
                                        phi_h[:, c * 128:(c + 1) * 128],
                                        theta_h[:, ntsl],
                                        start=True, stop=True,
                                    )
                                pt = pt_pool.tile([128, 3 * NT], BF16)
                                if g == n_grps - 1:
                                    # the last group's exp gates both the final
                                    # y01 mm2s and (via the S^T ring slot) the
                                    # next tile's first S^T: split its 2 chunks
                                    # across both engines to halve the latency
                                    nc.scalar.activation(pt[:, :NT], st[:, :NT], EXP)
                                    nc.vector.tensor_scalar(
                                        pt[:, NT:w].bitcast(I16), st[:, NT:w],
                                        FEXP_A, FEXP_B, op0=MULT, op1=ADD,
                                    )
                                elif g in DVE_GROUPS:
                                    nc.vector.tensor_scalar(
                                        pt[:, :w].bitcast(I16), st[:, :w],
                                        FEXP_A, FEXP_B, op0=MULT, op1=ADD,
                                    )
                                else:
                                    nc.scalar.activation(pt[:, :w], st[:, :w], EXP)
                                pts_cur.append(pt)
                                if g > 1:
                                    # y01 mm2 lags TWO groups behind S^T: exp(g)
                                    # then has ~2 pipeline positions before its
                                    # consumer instead of 1
                                    pc0, pc1 = GROUPS[g - 2]
                                    ia = mm2(y01[:, 0:O + 1], pts_cur[g - 2], 0, pc0, pc1, pc0)
                                    ib = mm2(y01[:, 130:259], pts_cur[g - 2], 1, pc0, pc1, pc0)
                                    if pc0 == 0:
                                        add_dep_helper(ib.ins, ia.ins, sync=False,
                                                       reason="bank-pack: clear before first write")
                            if g > 0:
                                for gg in y23_groups(g):
                                    c0, c1 = GROUPS[gg]
                                    ia = mm2(y23[:, 0:O + 1], pts_prev[gg], 2, c0, c1, c0)
                                    ib = mm2(y23[:, 130:259], pts_prev[gg], 3, c0, c1, c0)
                                    if c0 == 0:
                                        add_dep_helper(ib.ins, ia.ins, sync=False,
                                                       reason="bank-pack: clear before first write")
                            if g == 6:
                                if pv == last:
                                    # tail: first half of tile 7's output
                                    # projection straight after the final mm2
                                    # burst; its bn chains emitted immediately
                                    # (ct0 on DVE, ct1 on ScalarE) so they
                                    # overlap the norms/transposes
                                    for ai, args in enumerate(out_proj_mm_half(pv, 0, yts[pv])):
                                        bn_res_dma(*args, bn_eng=nc.vector if ai == 0 else None)
                                    norm_pair_pe(y23, yts[pv], 256, nc.scalar)
                                else:
                                    norm_transpose(y23[:, 0:O + 1], yts[pv], 256, nc.sync)
                                    norm_transpose(y23[:, 130:259], yts[pv], 384, nc.sync)
                            if g == 10:
                                if pv < last:
                                    bns += out_proj_mm_full(pv, yts[pv])
                                else:
                                    for ai, args in enumerate(out_proj_mm_half(pv, 1, yts[pv])):
                                        bn_res_dma(*args, bn_eng=nc.vector if ai == 0 else None)
                        if nt is not None:
                            for gg in (n_grps - 2, n_grps - 1):
                                c0, c1 = GROUPS[gg]
                                mm2(y01[:, 0:O + 1], pts_cur[gg], 0, c0, c1, c0)
                                mm2(y01[:, 130:259], pts_cur[gg], 1, c0, c1, c0)
                        for args in bns:
                            bn_res_dma(*args)
                        if nt is not None:
                            if nt == last:
                                # copies on DVE: scalar is still draining
                                # exp/bn residue here, and these copies gate h0
                                norm_pair_pe(y01, yts[nt], 0, nc.vector)
                            else:
                                norm_transpose(y01[:, 0:O + 1], yts[nt], 0, nc.sync)
                                norm_transpose(y01[:, 130:259], yts[nt], 128, nc.sync)
                        pts_prev = pts_cur

    nc.finalize()
    return nc


_NC_CACHE = None


def _get_nc():
    global _NC_CACHE
    if _NC_CACHE is None:
        _NC_CACHE = build_nc()
    return _NC_CACHE


def _prepare_in_maps(inputs):
    x = np.ascontiguousarray(np.asarray(inputs["x"], dtype=np.float32)).reshape(B, C, N)
    xh = x.astype(np.float16)
    wth = np.ascontiguousarray(np.asarray(inputs["w_theta"], np.float32).T).astype(np.float16)
    wph = np.ascontiguousarray(np.asarray(inputs["w_phi"], np.float32).T).astype(np.float16)
    wg = np.ascontiguousarray(np.asarray(inputs["w_g"], np.float32).T).astype(np.float16)
    w_out = np.asarray(inputs["w_out"], np.float32)
    wout = np.ascontiguousarray(w_out.T).astype(ml_dtypes.bfloat16)
    bth = np.asarray(inputs["b_theta"], np.float32).reshape(O, 1)
    bph = np.asarray(inputs["b_phi"], np.float32).reshape(O, 1)
    inv = np.asarray(inputs["bn_gamma"], np.float32) / np.sqrt(
        np.asarray(inputs["bn_var"], np.float32) + BN_EPS)
    shift = (np.asarray(inputs["b_out"], np.float32) * inv
             + np.asarray(inputs["bn_beta"], np.float32)
             - np.asarray(inputs["bn_mean"], np.float32) * inv)
    # fold the g-branch bias through the output projection: softmax rows sum
    # to 1, so attn @ (g + b_g) = attn @ g + b_g, and w_out @ b_g is constant
    wob = wout.astype(np.float32) .T @ np.asarray(inputs["b_g"], np.float32)
    shift = shift + inv * wob
    bnscale = np.ascontiguousarray(inv.reshape(2, 128).T)
    bnshift = np.ascontiguousarray(shift.reshape(2, 128).T)

    shared = dict(wth=wth, wph=wph, wg=wg, wout=wout, bth=bth, bph=bph,
                  bnscale=bnscale, bnshift=bnshift,
                  identm=np.eye(128, dtype=ml_dtypes.bfloat16))
    return [dict(shared, xin=np.ascontiguousarray(xh[b])) for b in range(B)]


def _install_ntff_shim():
    """This image's antenv lacks axon_hooks; provide it from trn_boot's
    ctypes implementation so trace=True can capture NTFF profiles."""
    import types
    try:
        import antenv.axon_hooks  # noqa: F401
        return
    except ImportError:
        pass
    if "/root/.axon_site" not in sys.path:
        sys.path.insert(0, "/root/.axon_site")
    from trn_agent_boot.trn_boot import _ntff_profile_via_ctypes
    hook = _ntff_profile_via_ctypes("/opt/axon/libaxon_pjrt.so")
    m = types.ModuleType("antenv.axon_hooks")
    m.get_axon_ntff_profile_hook = lambda: hook
    m.set_axon_ntff_profile_hook = lambda h: None
    sys.modules["antenv.axon_hooks"] = m


def run(inputs, trace=False):
    if trace:
        _install_ntff_shim()
    nc = _get_nc()
    in_maps = _prepare_in_maps(inputs)
    res = run_bass_kernel_spmd(nc, in_maps, list(range(B)), trace=trace)
    outs = np.stack([res.results[b]["out"] for b in range(B)])
    return outs.reshape(B, C, 64, 64), res


def kernel(**inputs) -> np.ndarray:
    out, _ = run(inputs)
    return out


if __name__ == "__main__":
    # quick CoreSim check of one core
    from concourse import bass_interp
    rng = np.random.default_rng(0)
    fake = {
        "x": rng.standard_normal((B, C, 64, 64)).astype(np.float32),
        "w_theta": (rng.standard_normal((O, C)) * 0.05).astype(np.float32),
        "b_theta": (rng.standard_normal(O) * 0.05).astype(np.float32),
        "w_phi": (rng.standard_normal((O, C)) * 0.05).astype(np.float32),
        "b_phi": (rng.standard_normal(O) * 0.05).astype(np.float32),
        "w_g": (rng.standard_normal((O, C)) * 0.05).astype(np.float32),
        "b_g": (rng.standard_normal(O) * 0.05).astype(np.float32),
        "w_out": (rng.standard_normal((C, O)) * 0.05).astype(np.float32),
        "b_out": (rng.standard_normal(C) * 0.05).astype(np.float32),
        "bn_gamma": rng.standard_normal(C).astype(np.float32),
        "bn_beta": rng.standard_normal(C).astype(np.float32),
        "bn_mean": rng.standard_normal(C).astype(np.float32),
        "bn_var": rng.uniform(0.5, 1.5, C).astype(np.float32),
    }
    nc = _get_nc()
    in_maps = _prepare_in_maps(fake)
    sim = bass_interp.CoreSim(nc)
    for k, v in in_maps[0].items():
        sim.tensor(k)[:] = v
    sim.simulate()
    got = np.asarray(sim.tensor("out"))

    x0 = fake["x"][0].reshape(C, N)
    th = fake["w_theta"] @ x0 + fake["b_theta"][:, None]
    ph = fake["w_phi"] @ x0 + fake["b_phi"][:, None]
    gg = fake["w_g"] @ x0 + fake["b_g"][:, None]
    s = th.T @ ph
    p = np.exp(s - s.max(1, keepdims=True))
    a = p / p.sum(1, keepdims=True)
    yy = a @ gg.T
    wy = fake["w_out"] @ yy.T + fake["b_out"][:, None]
    inv = fake["bn_gamma"] / np.sqrt(fake["bn_var"] + BN_EPS)
    bn = wy * inv[:, None] + (fake["bn_beta"] - fake["bn_mean"] * inv)[:, None]
    want = x0 + bn
    err = np.abs(got - want).max()
    print("CoreSim absmax err:", err, "rel:", err / np.abs(want).max())
